# revision 1
# baseline (speedup 1.0000x reference)
"""Trainium2 Bass kernel for ChunkedTGnnModel (2-layer GCN over temporal chunks).

Math: the reference flattens each temporal chunk to a [128000, 64] slab
(row u = node*128 + t_local) while edges are replicated per-timestep with
t-major offsets (tl*N + v). Both live in the same flat index space, so the
per-chunk operator is block-diagonal: 128 consecutive 1000-row blocks of the
slab each get the same dense normalized adjacency A_hat [1000 x 1000]:

    out = relu(blockdiag(A_hat) @ (slab @ W1) + b1)   (then layer 2 same)

Sharding: 8 cores = 4 chunks x 2 node-halves; each core owns a contiguous
[64000, 64] slab piece (64 blocks = 32 block-pairs).

Per block-pair (b, b+1), per layer, on-chip (all matmul operands fp16,
accumulation fp32 in PSUM):
  A-type:  G.T = lhsT.T @ AT  with lhsT = slab tiles [128 rho, 2x64 (blk,d)]
           -> feature-major G.T [128 (blk,d), 1000 dest] in PSUM
  W-fold:  lhsT = G.T chunk [128 feats, <=128 dest rows], rhs = blockdiag(W)
           -> row-major H [dest rows, 2x64 (blk,dout)] in PSUM
  epilogue: DVE bias add + ACT relu -> fp16 tiles (= next layer's lhsT)
"""
import sys
import numpy as np

sys.path.insert(0, '/opt/trn_rl_repo')

import concourse.bass as bass  # noqa: E402
import concourse.bacc as bacc  # noqa: E402
import concourse.mybir as mybir  # noqa: E402
import concourse.tile as tile  # noqa: E402
from concourse.bass_utils import run_bass_kernel_spmd  # noqa: E402

N, T, D = 1000, 512, 64
CS = 128                 # timesteps per chunk
NCORES = 8
ROWS = 64000             # slab rows per core (64 blocks x 1000)
PAIRS = 32
RHO = [(128 * j, min(128, N - 128 * j)) for j in range(8)]   # (start, rows)
DEST = [(0, 512), (512, 488)]                                # A-type dest chunks

_prog = None
LAST_RESULTS = None


def _build_program():
    nc = bacc.Bacc(None)
    xin = nc.declare_dram_parameter("xin", [ROWS, D], mybir.dt.float16, isOutput=False)
    at = nc.declare_dram_parameter("at", [N, N], mybir.dt.float16, isOutput=False)
    wt1 = nc.declare_dram_parameter("wt1", [128, 128], mybir.dt.float16, isOutput=False)
    wt2 = nc.declare_dram_parameter("wt2", [128, 128], mybir.dt.float16, isOutput=False)
    bs1 = nc.declare_dram_parameter("bs1", [128, 512], mybir.dt.float32, isOutput=False)
    bs2 = nc.declare_dram_parameter("bs2", [128, 512], mybir.dt.float32, isOutput=False)
    xout = nc.declare_dram_parameter("xout", [ROWS, D], mybir.dt.float32, isOutput=True)

    with tile.TileContext(nc) as tc:
        with tc.tile_pool(name="const", bufs=1) as cpool, \
             tc.tile_pool(name="work", bufs=2) as wpool, \
             tc.tile_pool(name="gps_pool", bufs=2, space="PSUM") as gpool, \
             tc.tile_pool(name="hps_pool", bufs=2, space="PSUM") as hpool:

            at_t = []
            for j, (r0, rj) in enumerate(RHO):
                t_ = cpool.tile([rj, N], mybir.dt.float16, name=f"at{j}")
                nc.sync.dma_start(t_[:, :], at[r0:r0 + rj, :])
                at_t.append(t_)
            wt_t = []
            for li, wsrc in enumerate((wt1, wt2)):
                w_ = cpool.tile([128, 128], mybir.dt.float16, name=f"wt{li}")
                nc.sync.dma_start(w_[:, :], wsrc[:, :])
                wt_t.append(w_)
            bs_t = []
            for li, bsrc in enumerate((bs1, bs2)):
                b_ = cpool.tile([128, 512], mybir.dt.float32, name=f"bst{li}")
                nc.sync.dma_start(b_[:, :], bsrc[:, :])
                bs_t.append(b_)

            def gcn_layer(p, li, lhsT_of, out_dtype):
                """One GCN layer on block-pair p. Returns 2 out tiles
                [128, 512] of dtype out_dtype, laid out [4 dchunks x 2 blk x 64]."""
                gps = []
                for ci, (c0, cw) in enumerate(DEST):
                    gp = gpool.tile([128, cw], mybir.dt.float32,
                                    name=f"gps{ci}", tag=f"gps{ci}")
                    for j, (r0, rj) in enumerate(RHO):
                        nc.tensor.matmul(gp[:, :], lhsT_of(j),
                                         at_t[j][:, c0:c0 + cw],
                                         start=(j == 0), stop=(j == 7))
                    gps.append(gp)
                gsb = wpool.tile([128, N], mybir.dt.float16,
                                 name=f"gsb{li}", tag=f"gsb{li}")
                nc.vector.tensor_copy(gsb[:, 0:512], gps[0][:, :])
                nc.vector.tensor_copy(gsb[:, 512:1000], gps[1][:, :])

                hps = [hpool.tile([128, 512], mybir.dt.float32,
                                  name=f"hps{t}", tag=f"hps{t}") for t in range(2)]
                for ci, (c0c, rci) in enumerate(RHO):
                    t_, o = ci // 4, 128 * (ci % 4)
                    nc.tensor.matmul(hps[t_][0:rci, o:o + 128],
                                     gsb[:, c0c:c0c + rci], wt_t[li][:, :],
                                     start=True, stop=True)
                outs = [wpool.tile([128, 512], out_dtype,
                                   name=f"ho{li}_{t}", tag=f"ho{li}_{t}")
                        for t in range(2)]
                for ci, (c0c, rci) in enumerate(RHO):
                    t_, o = ci // 4, 128 * (ci % 4)
                    nc.vector.tensor_add(hps[t_][0:rci, o:o + 128],
                                         hps[t_][0:rci, o:o + 128],
                                         bs_t[li][0:rci, o:o + 128])
                    nc.scalar.activation(outs[t_][0:rci, o:o + 128],
                                         hps[t_][0:rci, o:o + 128],
                                         mybir.ActivationFunctionType.Relu)
                return outs

            for p in range(PAIRS):
                blocks = (2 * p, 2 * p + 1)
                xt = []
                for j, (r0, rj) in enumerate(RHO):
                    t_ = wpool.tile([128, 128], mybir.dt.float16,
                                    name=f"xt{j}", tag=f"xt{j}")
                    for bi, blk in enumerate(blocks):
                        nc.sync.dma_start(
                            t_[0:rj, 64 * bi:64 * bi + 64],
                            xin[1000 * blk + r0:1000 * blk + r0 + rj, :])
                    xt.append(t_)

                h1 = gcn_layer(p, 0, lambda j: xt[j][0:RHO[j][1], :],
                               mybir.dt.float16)
                o2 = gcn_layer(p, 1,
                               lambda j: h1[j // 4][0:RHO[j][1],
                                                   128 * (j % 4):128 * (j % 4) + 128],
                               mybir.dt.float32)

                for ci, (c0c, rci) in enumerate(RHO):
                    t_, o = ci // 4, 128 * (ci % 4)
                    for bi, blk in enumerate(blocks):
                        nc.sync.dma_start(
                            xout[1000 * blk + c0c:1000 * blk + c0c + rci, :],
                            o2[t_][0:rci, o + 64 * bi:o + 64 * bi + 64])

    nc.compile()
    return nc


def _host_prep(x, edge_index, W1, b1, W2, b2):
    x = np.ascontiguousarray(np.asarray(x, dtype=np.float32))
    ei = np.asarray(edge_index)
    row, col = ei[0], ei[1]
    deg = np.zeros(N, np.float32)
    np.add.at(deg, col, 1.0)
    deg += 1.0
    dinv = (1.0 / np.sqrt(deg)).astype(np.float32)
    A = np.zeros((N, N), np.float32)
    np.add.at(A, (col, row), (dinv[row] * dinv[col]).astype(np.float32))
    A[np.arange(N), np.arange(N)] += dinv * dinv
    AT16 = np.ascontiguousarray(A.T).astype(np.float16)

    wts = []
    for W in (W1, W2):
        wt = np.zeros((128, 128), np.float16)
        wt[:64, :64] = np.asarray(W).astype(np.float16)
        wt[64:, 64:] = np.asarray(W).astype(np.float16)
        wts.append(wt)
    bss = [np.ascontiguousarray(
        np.broadcast_to(np.tile(np.asarray(b, np.float32), 8), (128, 512)))
        for b in (b1, b2)]

    x16 = x.astype(np.float16)
    slabs = []
    for k in range(NCORES):
        c, hf = k // 2, k % 2
        slab = np.ascontiguousarray(
            x16[500 * hf:500 * hf + 500, 128 * c:128 * (c + 1), :]).reshape(ROWS, D)
        slabs.append(slab)
    return AT16, wts, bss, slabs


def kernel(x, edge_index, W1, b1, W2, b2):
    global _prog, LAST_RESULTS
    if _prog is None:
        _prog = _build_program()
    nc = _prog

    AT16, wts, bss, slabs = _host_prep(x, edge_index, W1, b1, W2, b2)
    in_maps = [{"xin": slabs[k], "at": AT16,
                "wt1": wts[0], "wt2": wts[1],
                "bs1": bss[0], "bs2": bss[1]} for k in range(NCORES)]

    LAST_RESULTS = run_bass_kernel_spmd(nc, in_maps, core_ids=list(range(NCORES)))

    out = np.empty((N, T, D), np.float32)
    for k in range(NCORES):
        c, hf = k // 2, k % 2
        r = LAST_RESULTS.results[k]["xout"].reshape(500, CS, D)
        out[500 * hf:500 * hf + 500, 128 * c:128 * (c + 1), :] = r
    return out


# revision 11
# speedup vs baseline: 2.6748x; 2.6748x over previous
"""Trainium2 Bass kernel for ChunkedTGnnModel (2-layer GCN over temporal chunks).

Math: the reference flattens each temporal chunk to a [128000, 64] slab
(row u = node*128 + t_local) while edges are replicated per-timestep with
t-major offsets (tl*N + v). Both live in the same flat index space, so the
per-chunk operator is block-diagonal: 128 consecutive 1000-row blocks of the
slab each get the same dense normalized adjacency A_hat [1000 x 1000]:

    out = relu(blockdiag(A_hat) @ (slab @ W1) + b1)   (then layer 2 same)

Sharding: 8 cores = 4 chunks x 2 node-halves; each core owns a contiguous
[64000, 64] slab piece (64 blocks = 32 block-pairs).

Per block-pair (b, b+1), per layer, on-chip (all matmul operands fp16,
accumulation fp32 in PSUM):
  A-type:  G.T = lhsT.T @ AT  with lhsT = slab tiles [128 rho, 2x64 (blk,d)]
           -> feature-major G.T [128 (blk,d), 1000 dest] in PSUM
  W-fold:  lhsT = G.T chunk [128 feats, <=128 dest rows], rhs = blockdiag(W)
           -> row-major H [dest rows, 2x64 (blk,dout)] in PSUM
  epilogue: DVE bias add + ACT relu -> fp16 tiles (= next layer's lhsT)
"""
import sys
import numpy as np

sys.path.insert(0, '/opt/trn_rl_repo')

import concourse.bass as bass  # noqa: E402
import concourse.bacc as bacc  # noqa: E402
import concourse.mybir as mybir  # noqa: E402
import concourse.tile as tile  # noqa: E402
from concourse.bass_utils import run_bass_kernel_spmd  # noqa: E402

N, T, D = 1000, 512, 64
CS = 128                 # timesteps per chunk
NCORES = 8
ROWS = 64000             # slab rows per core (64 blocks x 1000)
PAIRS = 32
RHO = [(128 * j, min(128, N - 128 * j)) for j in range(8)]   # (start, rows)
DEST = [(0, 512), (512, 488)]                                # A-type dest chunks

_prog = None
LAST_RESULTS = None


def _build_program(skip=frozenset()):
    nc = bacc.Bacc(None)
    xin = nc.declare_dram_parameter("xin", [ROWS, D], mybir.dt.float16, isOutput=False)
    at = nc.declare_dram_parameter("at", [N, N], mybir.dt.float16, isOutput=False)
    wt1 = nc.declare_dram_parameter("wt1", [128, 128], mybir.dt.float16, isOutput=False)
    wt2 = nc.declare_dram_parameter("wt2", [128, 128], mybir.dt.float16, isOutput=False)
    bs1 = nc.declare_dram_parameter("bs1", [128, 512], mybir.dt.float32, isOutput=False)
    bs2 = nc.declare_dram_parameter("bs2", [128, 512], mybir.dt.float32, isOutput=False)
    xout = nc.declare_dram_parameter("xout", [ROWS, D], mybir.dt.float32, isOutput=True)

    with tile.TileContext(nc) as tc:
        with tc.tile_pool(name="const", bufs=1) as cpool, \
             tc.tile_pool(name="work", bufs=2) as wpool, \
             tc.tile_pool(name="gps_pool", bufs=2, space="PSUM") as gpool, \
             tc.tile_pool(name="hps_pool", bufs=2, space="PSUM") as hpool:

            # [64 blocks, 1000 rows, 64 feats] views of the slab in HBM
            xin_b = xin.rearrange("(blk r) d -> blk r d", r=N)
            xout_b = xout.rearrange("(blk r) d -> blk r d", r=N)

            # per-pair live state: xt/h1/o2 tiles, g psum tiles
            st = {}

            def load_xt(p):
                # xt_all column layout: col = j*128 + b*64 + d, so lhsT for
                # rho-tile j is the contiguous slice [128j, 128j+128).
                b0 = 2 * p
                xt_all = wpool.tile([128, 1024], mybir.dt.float16,
                                    name="xt_all", tag="xt_all")
                if "indma" not in skip:
                    for bi in range(2):
                        dst = xt_all.rearrange("p (j w) -> p j w", w=128)[
                            :, 0:7, 64 * bi:64 * bi + 64]
                        src = xin_b[b0 + bi, 0:896, :].rearrange(
                            "(j i) d -> i j d", j=7)
                        nc.sync.dma_start(dst, src)
                        nc.sync.dma_start(
                            xt_all[0:104, 896 + 64 * bi:896 + 64 * bi + 64],
                            xin_b[b0 + bi, 896:1000, :])
                st[p] = {"xt": xt_all}

            def stage_A(p, li):
                """A-type matmuls for layer li of pair p -> g psum tiles."""
                if li == 0:
                    xt_all = st[p]["xt"]
                    def lhsT_of(j):
                        return xt_all[0:RHO[j][1], 128 * j:128 * j + 128]
                else:
                    h1 = st[p]["h1"]
                    def lhsT_of(j):
                        return h1[j // 4][0:RHO[j][1],
                                          128 * (j % 4):128 * (j % 4) + 128]
                gps = []
                for ci, (c0, cw) in enumerate(DEST):
                    gp = gpool.tile([128, cw], mybir.dt.float32,
                                    name=f"gps{ci}", tag=f"gps{ci}")
                    if "atype" not in skip:
                        for j, (r0, rj) in enumerate(RHO):
                            nc.tensor.matmul(gp[:, :], lhsT_of(j),
                                             at_t[j][:, c0:c0 + cw],
                                             start=(j == 0), stop=(j == 7))
                    gps.append(gp)
                st[p][f"g{li}"] = gps

            def stage_W(p, li):
                """psum->sbuf copies, W-fold matmuls, bias+relu for layer li."""
                gps = st[p].pop(f"g{li}")
                out_dtype = mybir.dt.float16 if li == 0 else mybir.dt.float32
                gsb = wpool.tile([128, N], mybir.dt.float16,
                                 name=f"gsb{li}", tag=f"gsb{li}")
                if "copies" not in skip:
                    # split across ACT and DVE to balance engine load
                    nc.scalar.copy(gsb[:, 0:512], gps[0][:, :])
                    nc.vector.tensor_copy(gsb[:, 512:1000], gps[1][:, :])

                hps = [hpool.tile([128, 512], mybir.dt.float32,
                                  name=f"hps{t}", tag=f"hps{t}") for t in range(2)]
                if "wfold" not in skip:
                    for ci, (c0c, rci) in enumerate(RHO):
                        t_, o = ci // 4, 128 * (ci % 4)
                        nc.tensor.matmul(hps[t_][0:rci, o:o + 128],
                                         gsb[:, c0c:c0c + rci], wt_t[li][:, :],
                                         start=True, stop=True)
                outs = [wpool.tile([128, 512], out_dtype,
                                   name=f"ho{li}_{t}", tag=f"ho{li}_{t}")
                        for t in range(2)]
                # coarse epilogue regions: (bank, rows, col0, col1); the last
                # delta-chunk only has 104 valid rows so it gets its own op.
                regions = [(0, 128, 0, 512), (1, 128, 0, 384), (1, 104, 384, 512)]
                for t_, rr, c0r, c1r in regions:
                    if "bias" not in skip:
                        nc.vector.tensor_add(hps[t_][0:rr, c0r:c1r],
                                             hps[t_][0:rr, c0r:c1r],
                                             bs_t[li][0:rr, 0:c1r - c0r])
                    if "relu" not in skip:
                        nc.scalar.activation(outs[t_][0:rr, c0r:c1r],
                                             hps[t_][0:rr, c0r:c1r],
                                             mybir.ActivationFunctionType.Relu)
                st[p]["h1" if li == 0 else "o2"] = outs

            def store_out(p):
                o2 = st[p].pop("o2")
                b0 = 2 * p
                if "outdma" not in skip:
                    o2v = [t_.rearrange("p (c b d) -> p c b d", c=4, b=2)
                           for t_ in o2]
                    for bi in range(2):
                        # split 3 on SWDGE (Pool) / 3 on HWDGE (SP)
                        eng = nc.gpsimd if bi == 0 else nc.sync
                        dstA = xout_b[b0 + bi, 0:512, :].rearrange(
                            "(c i) d -> i c d", c=4)
                        eng.dma_start(dstA, o2v[0][:, :, bi, :])
                        dstB = xout_b[b0 + bi, 512:896, :].rearrange(
                            "(c i) d -> i c d", c=3)
                        eng.dma_start(dstB, o2v[1][:, 0:3, bi, :])
                        eng.dma_start(
                            xout_b[b0 + bi, 896:1000, :],
                            o2v[1][0:104, 3, bi, :])
                del st[p]

            # prologue: first pair's input before the constants so the first
            # A-type matmuls start as early as possible
            # constants go over the Pool/SWDGE path so they don't serialize
            # against the pair input loads on HWDGE
            load_xt(0)
            at_t = []
            for j, (r0, rj) in enumerate(RHO):
                t_ = cpool.tile([rj, N], mybir.dt.float16, name=f"at{j}")
                nc.gpsimd.dma_start(t_[:, :], at[r0:r0 + rj, :])
                at_t.append(t_)
            wt_t = []
            for li, wsrc in enumerate((wt1, wt2)):
                w_ = cpool.tile([128, 128], mybir.dt.float16, name=f"wt{li}")
                nc.gpsimd.dma_start(w_[:, :], wsrc[:, :])
                wt_t.append(w_)
            bs_t = []
            for li, bsrc in enumerate((bs1, bs2)):
                b_ = cpool.tile([128, 512], mybir.dt.float32, name=f"bst{li}")
                nc.gpsimd.dma_start(b_[:, :], bsrc[:, :])
                bs_t.append(b_)

            stage_A(0, 0)
            # software-pipelined steady state: every PE stall window is
            # covered by >=3us of independent A-type work from another pair
            for p in range(PAIRS + 1):
                if p + 1 < PAIRS:
                    load_xt(p + 1)
                if p < PAIRS:
                    stage_W(p, 0)
                if p + 1 < PAIRS:
                    stage_A(p + 1, 0)
                if p >= 1:
                    stage_W(p - 1, 1)
                    store_out(p - 1)
                if p < PAIRS:
                    stage_A(p, 1)

    nc.compile()
    return nc


def _host_prep(x, edge_index, W1, b1, W2, b2):
    x = np.ascontiguousarray(np.asarray(x, dtype=np.float32))
    ei = np.asarray(edge_index)
    row, col = ei[0], ei[1]
    deg = np.zeros(N, np.float32)
    np.add.at(deg, col, 1.0)
    deg += 1.0
    dinv = (1.0 / np.sqrt(deg)).astype(np.float32)
    A = np.zeros((N, N), np.float32)
    np.add.at(A, (col, row), (dinv[row] * dinv[col]).astype(np.float32))
    A[np.arange(N), np.arange(N)] += dinv * dinv
    AT16 = np.ascontiguousarray(A.T).astype(np.float16)

    wts = []
    for W in (W1, W2):
        wt = np.zeros((128, 128), np.float16)
        wt[:64, :64] = np.asarray(W).astype(np.float16)
        wt[64:, 64:] = np.asarray(W).astype(np.float16)
        wts.append(wt)
    bss = [np.ascontiguousarray(
        np.broadcast_to(np.tile(np.asarray(b, np.float32), 8), (128, 512)))
        for b in (b1, b2)]

    x16 = x.astype(np.float16)
    slabs = []
    for k in range(NCORES):
        c, hf = k // 2, k % 2
        slab = np.ascontiguousarray(
            x16[500 * hf:500 * hf + 500, 128 * c:128 * (c + 1), :]).reshape(ROWS, D)
        slabs.append(slab)
    return AT16, wts, bss, slabs


def kernel(x, edge_index, W1, b1, W2, b2):
    global _prog, LAST_RESULTS
    if _prog is None:
        _prog = _build_program()
    nc = _prog

    AT16, wts, bss, slabs = _host_prep(x, edge_index, W1, b1, W2, b2)
    in_maps = [{"xin": slabs[k], "at": AT16,
                "wt1": wts[0], "wt2": wts[1],
                "bs1": bss[0], "bs2": bss[1]} for k in range(NCORES)]

    LAST_RESULTS = run_bass_kernel_spmd(nc, in_maps, core_ids=list(range(NCORES)))

    out = np.empty((N, T, D), np.float32)
    for k in range(NCORES):
        c, hf = k // 2, k % 2
        r = LAST_RESULTS.results[k]["xout"].reshape(500, CS, D)
        out[500 * hf:500 * hf + 500, 128 * c:128 * (c + 1), :] = r
    return out


# revision 12
# speedup vs baseline: 2.6994x; 1.0092x over previous
"""Trainium2 Bass kernel for ChunkedTGnnModel (2-layer GCN over temporal chunks).

Math: the reference flattens each temporal chunk to a [128000, 64] slab
(row u = node*128 + t_local) while edges are replicated per-timestep with
t-major offsets (tl*N + v). Both live in the same flat index space, so the
per-chunk operator is block-diagonal: 128 consecutive 1000-row blocks of the
slab each get the same dense normalized adjacency A_hat [1000 x 1000]:

    out = relu(blockdiag(A_hat) @ (slab @ W1) + b1)   (then layer 2 same)

Sharding: 8 cores = 4 chunks x 2 node-halves; each core owns a contiguous
[64000, 64] slab piece (64 blocks = 32 block-pairs).

Per block-pair (b, b+1), per layer, on-chip (all matmul operands fp16,
accumulation fp32 in PSUM):
  A-type:  G.T = lhsT.T @ AT  with lhsT = slab tiles [128 rho, 2x64 (blk,d)]
           -> feature-major G.T [128 (blk,d), 1000 dest] in PSUM
  W-fold:  lhsT = G.T chunk [128 feats, <=128 dest rows], rhs = blockdiag(W)
           -> row-major H [dest rows, 2x64 (blk,dout)] in PSUM
  epilogue: DVE bias add + ACT relu -> fp16 tiles (= next layer's lhsT)
"""
import sys
import numpy as np

sys.path.insert(0, '/opt/trn_rl_repo')

import concourse.bass as bass  # noqa: E402
import concourse.bacc as bacc  # noqa: E402
import concourse.mybir as mybir  # noqa: E402
import concourse.tile as tile  # noqa: E402
from concourse.bass_utils import run_bass_kernel_spmd  # noqa: E402

N, T, D = 1000, 512, 64
CS = 128                 # timesteps per chunk
NCORES = 8
ROWS = 64000             # slab rows per core (64 blocks x 1000)
PAIRS = 32
RHO = [(128 * j, min(128, N - 128 * j)) for j in range(8)]   # (start, rows)
DEST = [(0, 512), (512, 488)]                                # A-type dest chunks

_prog = None
LAST_RESULTS = None


def _build_program(skip=frozenset()):
    nc = bacc.Bacc(None)
    xin = nc.declare_dram_parameter("xin", [ROWS, D], mybir.dt.float16, isOutput=False)
    at = nc.declare_dram_parameter("at", [N, N], mybir.dt.float16, isOutput=False)
    wt1 = nc.declare_dram_parameter("wt1", [128, 128], mybir.dt.float16, isOutput=False)
    wt2 = nc.declare_dram_parameter("wt2", [128, 128], mybir.dt.float16, isOutput=False)
    bs1 = nc.declare_dram_parameter("bs1", [128, 512], mybir.dt.float32, isOutput=False)
    bs2 = nc.declare_dram_parameter("bs2", [128, 512], mybir.dt.float32, isOutput=False)
    xout = nc.declare_dram_parameter("xout", [ROWS, D], mybir.dt.float32, isOutput=True)

    with tile.TileContext(nc) as tc:
        with tc.tile_pool(name="const", bufs=1) as cpool, \
             tc.tile_pool(name="work", bufs=2) as wpool, \
             tc.tile_pool(name="gps_pool", bufs=2, space="PSUM") as gpool, \
             tc.tile_pool(name="hps_pool", bufs=2, space="PSUM") as hpool:

            # [64 blocks, 1000 rows, 64 feats] views of the slab in HBM
            xin_b = xin.rearrange("(blk r) d -> blk r d", r=N)
            xout_b = xout.rearrange("(blk r) d -> blk r d", r=N)

            # per-pair live state: xt/h1/o2 tiles, g psum tiles
            st = {}

            def load_xt(p):
                # xt_all column layout: col = j*128 + b*64 + d, so lhsT for
                # rho-tile j is the contiguous slice [128j, 128j+128).
                b0 = 2 * p
                xt_all = wpool.tile([128, 1024], mybir.dt.float16,
                                    name="xt_all", tag="xt_all")
                if "indma" not in skip:
                    for bi in range(2):
                        dst = xt_all.rearrange("p (j w) -> p j w", w=128)[
                            :, 0:7, 64 * bi:64 * bi + 64]
                        src = xin_b[b0 + bi, 0:896, :].rearrange(
                            "(j i) d -> i j d", j=7)
                        nc.sync.dma_start(dst, src)
                        nc.sync.dma_start(
                            xt_all[0:104, 896 + 64 * bi:896 + 64 * bi + 64],
                            xin_b[b0 + bi, 896:1000, :])
                st[p] = {"xt": xt_all}

            def stage_A(p, li):
                """A-type matmuls for layer li of pair p -> g psum tiles."""
                if li == 0:
                    xt_all = st[p]["xt"]
                    def lhsT_of(j):
                        return xt_all[0:RHO[j][1], 128 * j:128 * j + 128]
                else:
                    h1 = st[p]["h1"]
                    def lhsT_of(j):
                        return h1[j // 4][0:RHO[j][1],
                                          128 * (j % 4):128 * (j % 4) + 128]
                gps = []
                for ci, (c0, cw) in enumerate(DEST):
                    gp = gpool.tile([128, cw], mybir.dt.float32,
                                    name=f"gps{ci}", tag=f"gps{ci}")
                    if "atype" not in skip:
                        for j, (r0, rj) in enumerate(RHO):
                            nc.tensor.matmul(gp[:, :], lhsT_of(j),
                                             at_t[j][:, c0:c0 + cw],
                                             start=(j == 0), stop=(j == 7))
                    gps.append(gp)
                st[p][f"g{li}"] = gps

            def stage_W(p, li):
                """psum->sbuf copies, W-fold matmuls, bias+relu for layer li."""
                gps = st[p].pop(f"g{li}")
                out_dtype = mybir.dt.float16 if li == 0 else mybir.dt.float32
                gsb = wpool.tile([128, N], mybir.dt.float16,
                                 name=f"gsb{li}", tag=f"gsb{li}")
                if "copies" not in skip:
                    # split across ACT and DVE to balance engine load
                    nc.scalar.copy(gsb[:, 0:512], gps[0][:, :])
                    nc.vector.tensor_copy(gsb[:, 512:1000], gps[1][:, :])

                hps = [hpool.tile([128, 512], mybir.dt.float32,
                                  name=f"hps{t}", tag=f"hps{t}") for t in range(2)]
                if "wfold" not in skip:
                    for ci, (c0c, rci) in enumerate(RHO):
                        t_, o = ci // 4, 128 * (ci % 4)
                        nc.tensor.matmul(hps[t_][0:rci, o:o + 128],
                                         gsb[:, c0c:c0c + rci], wt_t[li][:, :],
                                         start=True, stop=True)
                outs = [wpool.tile([128, 512], out_dtype,
                                   name=f"ho{li}_{t}", tag=f"ho{li}_{t}")
                        for t in range(2)]
                # coarse epilogue regions: (bank, rows, col0, col1); the last
                # delta-chunk only has 104 valid rows so it gets its own op.
                regions = [(0, 128, 0, 512), (1, 128, 0, 384), (1, 104, 384, 512)]
                for t_, rr, c0r, c1r in regions:
                    if "bias" not in skip:
                        nc.vector.tensor_add(hps[t_][0:rr, c0r:c1r],
                                             hps[t_][0:rr, c0r:c1r],
                                             bs_t[li][0:rr, 0:c1r - c0r])
                    if "relu" not in skip:
                        nc.scalar.activation(outs[t_][0:rr, c0r:c1r],
                                             hps[t_][0:rr, c0r:c1r],
                                             mybir.ActivationFunctionType.Relu)
                st[p]["h1" if li == 0 else "o2"] = outs

            def store_out(p):
                o2 = st[p].pop("o2")
                b0 = 2 * p
                if "outdma" not in skip:
                    o2v = [t_.rearrange("p (c b d) -> p c b d", c=4, b=2)
                           for t_ in o2]
                    for bi in range(2):
                        # split 3 on SWDGE (Pool) / 3 on HWDGE (SP)
                        eng = nc.gpsimd if bi == 0 else nc.sync
                        dstA = xout_b[b0 + bi, 0:512, :].rearrange(
                            "(c i) d -> i c d", c=4)
                        eng.dma_start(dstA, o2v[0][:, :, bi, :])
                        dstB = xout_b[b0 + bi, 512:896, :].rearrange(
                            "(c i) d -> i c d", c=3)
                        eng.dma_start(dstB, o2v[1][:, 0:3, bi, :])
                        eng.dma_start(
                            xout_b[b0 + bi, 896:1000, :],
                            o2v[1][0:104, 3, bi, :])
                del st[p]

            # prologue: first pair's input before the constants so the first
            # A-type matmuls start as early as possible
            # constants go over the Pool/SWDGE path so they don't serialize
            # against the pair input loads on HWDGE
            load_xt(0)
            at_t = []
            for j, (r0, rj) in enumerate(RHO):
                t_ = cpool.tile([rj, N], mybir.dt.float16, name=f"at{j}")
                eng = nc.gpsimd if j % 2 == 0 else nc.sync
                eng.dma_start(t_[:, :], at[r0:r0 + rj, :])
                at_t.append(t_)
            wt_t = []
            for li, wsrc in enumerate((wt1, wt2)):
                w_ = cpool.tile([128, 128], mybir.dt.float16, name=f"wt{li}")
                nc.gpsimd.dma_start(w_[:, :], wsrc[:, :])
                wt_t.append(w_)
            bs_t = []
            for li, bsrc in enumerate((bs1, bs2)):
                b_ = cpool.tile([128, 512], mybir.dt.float32, name=f"bst{li}")
                nc.gpsimd.dma_start(b_[:, :], bsrc[:, :])
                bs_t.append(b_)

            stage_A(0, 0)
            # software-pipelined steady state: every PE stall window is
            # covered by >=3us of independent A-type work from another pair
            for p in range(PAIRS + 1):
                if p + 1 < PAIRS:
                    load_xt(p + 1)
                if p < PAIRS:
                    stage_W(p, 0)
                if p + 1 < PAIRS:
                    stage_A(p + 1, 0)
                if p >= 1:
                    stage_W(p - 1, 1)
                    store_out(p - 1)
                if p < PAIRS:
                    stage_A(p, 1)

    nc.compile()
    return nc


def _host_prep(x, edge_index, W1, b1, W2, b2):
    x = np.ascontiguousarray(np.asarray(x, dtype=np.float32))
    ei = np.asarray(edge_index)
    row, col = ei[0], ei[1]
    deg = np.zeros(N, np.float32)
    np.add.at(deg, col, 1.0)
    deg += 1.0
    dinv = (1.0 / np.sqrt(deg)).astype(np.float32)
    A = np.zeros((N, N), np.float32)
    np.add.at(A, (col, row), (dinv[row] * dinv[col]).astype(np.float32))
    A[np.arange(N), np.arange(N)] += dinv * dinv
    AT16 = np.ascontiguousarray(A.T).astype(np.float16)

    wts = []
    for W in (W1, W2):
        wt = np.zeros((128, 128), np.float16)
        wt[:64, :64] = np.asarray(W).astype(np.float16)
        wt[64:, 64:] = np.asarray(W).astype(np.float16)
        wts.append(wt)
    bss = [np.ascontiguousarray(
        np.broadcast_to(np.tile(np.asarray(b, np.float32), 8), (128, 512)))
        for b in (b1, b2)]

    x16 = x.astype(np.float16)
    slabs = []
    for k in range(NCORES):
        c, hf = k // 2, k % 2
        slab = np.ascontiguousarray(
            x16[500 * hf:500 * hf + 500, 128 * c:128 * (c + 1), :]).reshape(ROWS, D)
        slabs.append(slab)
    return AT16, wts, bss, slabs


def kernel(x, edge_index, W1, b1, W2, b2):
    global _prog, LAST_RESULTS
    if _prog is None:
        _prog = _build_program()
    nc = _prog

    AT16, wts, bss, slabs = _host_prep(x, edge_index, W1, b1, W2, b2)
    in_maps = [{"xin": slabs[k], "at": AT16,
                "wt1": wts[0], "wt2": wts[1],
                "bs1": bss[0], "bs2": bss[1]} for k in range(NCORES)]

    LAST_RESULTS = run_bass_kernel_spmd(nc, in_maps, core_ids=list(range(NCORES)))

    out = np.empty((N, T, D), np.float32)
    for k in range(NCORES):
        c, hf = k // 2, k % 2
        r = LAST_RESULTS.results[k]["xout"].reshape(500, CS, D)
        out[500 * hf:500 * hf + 500, 128 * c:128 * (c + 1), :] = r
    return out
